# revision 12
# baseline (speedup 1.0000x reference)
"""MoE top-1 feed-forward (DeepSpeed-style) on 8 Trainium2 NeuronCores.

Strategy (expert parallelism, per the sharding hint):
  - Host computes the (tiny) gate: logits = x @ Wg, softmax, top-1 expert id
    and gate prob per token (float64 for a faithful argmax).
  - Tokens are dispatched to the core owning their expert (core e holds
    W1[e]/b1[e]/W2[e]/b2[e]); each core's token batch is padded to a common
    capacity C so all 8 cores run one SPMD program.
  - Each core runs the dense FFN for its tokens in bf16 (tolerance is 2e-2;
    bf16 keeps rel-err ~3e-3 while halving HBM traffic and enabling the
    PE's fast-weight-load path, which fp32 weights cannot use):
        hT = silu(W1^T @ xT + b1);  yT = W2^T @ hT
    with tokens along the free (moving) dimension so no device transposes.
  - Schedule: x is split across BOTH HWDGE rings as the first two DMAs so
    the PE's critical-path input lands before the weight stream saturates
    HBM; a short burst of dummy matmuls on memset scratch warms the PE HAM
    clock-gate during the DMA fill so real matmuls run at 2.4 GHz from the
    start; W1/W2 stream in a few large group DMAs (small leading groups so
    compute starts early); y is evacuated as bf16 and stored in one DMA.
  - Host combines: out[token] = gate * (y + b2[expert]).
"""

import os
import sys

import numpy as np

try:
    import concourse.mybir as mybir  # noqa: F401
except ModuleNotFoundError:  # fallback if the site hooks aren't installed
    sys.path.insert(0, "/opt/trn_rl_repo")

import concourse.mybir as mybir
import concourse.tile as tile
from concourse import bacc
from concourse.bass_utils import run_bass_kernel_spmd

N_CORES = 8

# Compute dtype for the matmuls ("bf16" | "f32r" | "f32").
MODE = os.environ.get("BASS_MOE_MODE", "bf16")

# F-chunk widths per W1 / W2 group DMA (small leads -> compute starts early).
W1G = [int(s) for s in os.environ.get("BASS_MOE_W1G", "1,2,3,6,6,6").split(",")]
W2G = [int(s) for s in os.environ.get("BASS_MOE_W2G", "2,4,6,6,6").split(",")]
NDUM = int(os.environ.get("BASS_MOE_NDUM", "12"))  # PE warm-up matmuls

RAW = os.environ.get("BASS_MOE_RAW", "0") == "1"

_CACHE: dict = {}


def _roundup(a: int, m: int) -> int:
    return -(-a // m) * m


def _build_bass_raw(C: int, mode: str, D: int, F: int):
    """Raw bacc build (no TileContext): hand-rolled semaphores.

    Tile allocates ~254 semaphores and its exit drain serially resets every
    one (~8us of measured kernel time).  This version uses ~18 semaphores
    with explicit waits, full-resident weight buffers (no slot recycling,
    so no WAR waits on weight tiles), and a ~1us tail.

    Semaphore-safety note: a semaphore shared by several HWDGE DMAs may only
    be waited at its FINAL value — each DMA's 16 per-SDMA-engine increments
    interleave across transfers, so partial thresholds can be reached before
    any single transfer is complete.  Hence one semaphore per weight-group
    DMA, each waited at exactly 16.
    """
    import contextlib

    f32 = mybir.dt.float32
    if mode == "bf16":
        dt_io = mybir.dt.bfloat16
    elif mode == "f32r":
        dt_io = mybir.dt.float32r
    else:
        dt_io = f32

    KD, KF = D // 128, F // 128
    assert sum(W1G) == KF and sum(W2G) == KF and C <= 512
    DH = (KD // 2) * C

    nc = bacc.Bacc(None, target_bir_lowering=False, debug=False)
    xa = nc.dram_tensor("xa", [128, DH], dt_io, kind="ExternalInput")
    xb = nc.dram_tensor("xb", [128, KD * C - DH], dt_io, kind="ExternalInput")
    w1 = nc.dram_tensor("w1", [128, KD * F], dt_io, kind="ExternalInput")
    w2 = nc.dram_tensor("w2", [128, KF * D], dt_io, kind="ExternalInput")
    b1r = nc.dram_tensor("b1r", [128, KF], f32, kind="ExternalInput")
    yo = nc.dram_tensor("yo", [128, KD * C], dt_io, kind="ExternalOutput")

    silu = mybir.ActivationFunctionType.Silu
    es = contextlib.ExitStack()
    nc._moe_ctx = es  # keep allocations alive for the life of the program

    sem = lambda name: es.enter_context(nc.semaphore(name))
    s_x = sem("s_x")
    s_b1 = sem("s_b1")
    s_w1 = [sem(f"s_w1_{g}") for g in range(len(W1G))]
    s_w2 = [sem(f"s_w2_{g}") for g in range(len(W2G))]
    s_dum = sem("s_dum")
    s_ph = sem("s_ph")
    s_act = sem("s_act")
    s_mm2 = sem("s_mm2")
    s_fin = sem("s_fin")
    s_dve = sem("s_dve")
    s_yo = sem("s_yo")
    all_sems = [s_x, s_b1, *s_w1, *s_w2, s_dum, s_ph, s_act, s_mm2, s_fin, s_dve, s_yo]

    sb = lambda name, shape, dt: es.enter_context(nc.sbuf_tensor(name, shape, dt))
    ps = lambda name, shape: es.enter_context(nc.psum_tensor(name, shape, f32))
    xw = sb("xw", [128, KD * C], dt_io)
    w1b = sb("w1b", [128, KD * F], dt_io)
    w2b = sb("w2b", [128, KF * D], dt_io)
    b1t = sb("b1t", [128, KF], f32)
    htb = sb("htb", [128, 4 * C], dt_io)  # 4-slot ring of hT chunks
    yt = sb("yt", [128, KD * C], dt_io)
    wsc = sb("wsc", [128, 128], dt_io)
    xsc = sb("xsc", [128, C], dt_io)
    ph = [ps("ph0", [128, C]), ps("ph1", [128, C])]
    py = [ps(f"py{dd}", [128, C]) for dd in range(KD)]

    ht = lambda f: htb[:, (f % 4) * C : (f % 4 + 1) * C]

    # --- DVE: scratch memsets (unblock PE warm-up), later PSUM evacuation
    nc.vector.memset(wsc[:], 0.0).then_inc(s_dum, 1)
    nc.vector.memset(xsc[:], 0.0).then_inc(s_dum, 1)

    # --- scalar ring carries ONLY xb + b1: more DMA issues here would
    # occupy the ACT queue and stall the first silu (and with it mm2(0)).
    nc.scalar.dma_start(out=xw[:, DH:], in_=xb[:]).then_inc(s_x, 16)
    nc.scalar.dma_start(out=b1t[:], in_=b1r[:]).then_inc(s_b1, 16)

    # --- sync ring: x half first, then the W1 stream
    nc.sync.dma_start(out=xw[:, :DH], in_=xa[:]).then_inc(s_x, 16)
    f0 = 0
    for g, gw in enumerate(W1G):
        off = KD * 128 * f0
        sz = KD * gw * 128
        nc.sync.dma_start(
            out=w1b[:, off : off + sz], in_=w1[:, off : off + sz]
        ).then_inc(s_w1[g], 16)
        f0 += gw

    # --- W2 stream via SWDGE on the otherwise-idle GpSimd engine: shares
    # the same 16 SDMA engines (so full aggregate bandwidth) without
    # stealing issue slots from the sync or ACT queues.
    f0 = 0
    for g, gw in enumerate(W2G):
        nc.gpsimd.dma_start(
            out=w2b[:, f0 * D : (f0 + gw) * D], in_=w2[:, f0 * D : (f0 + gw) * D]
        ).then_inc(s_w2[g], 16)
        f0 += gw

    w1_of_f, w2_of_f = {}, {}
    w1_f0, w2_f0 = {}, {}
    f0 = 0
    for g, gw in enumerate(W1G):
        for j in range(gw):
            w1_of_f[f0 + j] = g
            w1_f0[f0 + j] = f0
        f0 += gw
    f0 = 0
    for g, gw in enumerate(W2G):
        for j in range(gw):
            w2_of_f[f0 + j] = g
        f0 += gw

    # --- PE program
    if NDUM > 0:
        nc.tensor.wait_ge(s_dum, 2)
        for _ in range(NDUM):
            nc.tensor.matmul(ph[0][:], wsc[:], xsc[:], start=True, stop=True)

    def emit_mm2(f):
        g = w2_of_f[f]
        nc.tensor.wait_ge(s_act, f + 1)  # ht(f) written
        nc.tensor.wait_ge(s_w2[g], 16)
        for dd in range(KD):
            mm = nc.tensor.matmul(
                py[dd][:],
                w2b[:, f * D + dd * 128 : f * D + (dd + 1) * 128],
                ht(f),
                start=(f == 0),
                stop=(f == KF - 1),
            )
            if f == KF - 1:
                mm.then_inc(s_fin, 1)
            elif dd == KD - 1:
                mm.then_inc(s_mm2, 1)

    seen_w1 = set()
    for f in range(KF):
        g = w1_of_f[f]
        if g not in seen_w1:
            seen_w1.add(g)
            nc.tensor.wait_ge(s_w1[g], 16)
        if f == 0:
            nc.tensor.wait_ge(s_x, 32)
        if f >= 2:
            nc.tensor.wait_ge(s_act, f - 1)  # ph slot (f-2) consumed
        gw = W1G[g]
        j = f - w1_f0[f]
        off = KD * 128 * w1_f0[f]
        for d in range(KD):
            mm = nc.tensor.matmul(
                ph[f % 2][:],
                w1b[:, off + (d * gw + j) * 128 : off + (d * gw + j + 1) * 128],
                xw[:, d * C : (d + 1) * C],
                start=(d == 0),
                stop=(d == KD - 1),
            )
            if d == KD - 1:
                mm.then_inc(s_ph, 1)
        if f >= 1:
            emit_mm2(f - 1)
    emit_mm2(KF - 1)

    # --- ACT program (after the scalar-ring DMA issues above)
    for f in range(KF):
        nc.scalar.wait_ge(s_ph, f + 1)
        if f == 0:
            nc.scalar.wait_ge(s_b1, 16)
        if f >= 4:
            nc.scalar.wait_ge(s_mm2, f - 3)  # ht slot (f-4) consumed by mm2
        nc.scalar.activation(
            ht(f), ph[f % 2][:], silu, bias=b1t[:, f : f + 1]
        ).then_inc(s_act, 1)

    # --- DVE evacuation (casts f32 PSUM -> dt_io); each copy chases the
    # matching final-chunk matmul so the first half of y can ship early
    for dd in range(KD):
        nc.vector.wait_ge(s_fin, dd + 1)
        nc.vector.tensor_copy(yt[:, dd * C : (dd + 1) * C], py[dd][:]).then_inc(
            s_dve, 1
        )

    # --- y out split across both rings: first half overlaps the last
    # matmuls/casts, second half follows immediately
    half = KD // 2
    nc.sync.wait_ge(s_dve, half)
    nc.sync.dma_start(out=yo[:, : half * C], in_=yt[:, : half * C]).then_inc(s_yo, 16)
    nc.scalar.wait_ge(s_dve, KD)
    nc.scalar.dma_start(out=yo[:, half * C :], in_=yt[:, half * C :]).then_inc(
        s_yo, 16
    )
    nc.sync.wait_ge(s_yo, 32)

    # --- exit: barrier, then reset our semaphores so the NEFF can re-run
    nc.all_engine_barrier()
    nums = sorted(s.num for s in all_sems)
    lo = 0
    while lo < len(nums):
        hi = lo
        while hi + 1 < len(nums) and nums[hi + 1] == nums[hi] + 1:
            hi += 1
        r = range(nums[lo], nums[hi] + 1)
        nc.gpsimd.dma_reset(r)
        nc.gpsimd.sem_clear(r)
        lo = hi + 1
    nc.all_engine_barrier()

    nc.compile()
    return nc


def _build_bass(C: int, mode: str, D: int, F: int):
    """Build + compile the per-core Bass program for capacity C (<= 512)."""
    f32 = mybir.dt.float32
    if mode == "bf16":
        dt_io = mybir.dt.bfloat16
    elif mode == "f32r":
        dt_io = mybir.dt.float32r
    else:
        dt_io = f32
    esz = 2 if mode == "bf16" else 4

    KD, KF = D // 128, F // 128
    assert sum(W1G) == KF and sum(W2G) == KF
    assert C <= 512
    DH = (KD // 2) * C  # x half-image width (d-chunks 0..KD/2-1)

    nc = bacc.Bacc(None, target_bir_lowering=False, debug=False)
    # Host-packed images (see kernel() for packing). Two HWDGE rings: sync
    # carries xa + W1 + y-out, scalar carries xb + b1 + W2.
    #   xa/xb [128, DH]     x^T halves: [p, d*C+c] = x[c, d*128+p]
    #   w1    [128, KD*F]   flat group images; group g at column offset
    #                       KD*128*sum(W1G[:g]), block (d, j) at (d*gw+j)*128
    #   w2    [128, KF*D]   flat image: [p, f*D+j] = W2[f*128+p, j]
    #   b1r   [128, KF]     b1[f*128+p] at [p, f]
    #   yo    [128, KD*C]   output (dt_io): [p, d*C+c] = y[c, d*128+p]
    xa = nc.dram_tensor("xa", [128, DH], dt_io, kind="ExternalInput")
    xb = nc.dram_tensor("xb", [128, KD * C - DH], dt_io, kind="ExternalInput")
    w1 = nc.dram_tensor("w1", [128, KD * F], dt_io, kind="ExternalInput")
    w2 = nc.dram_tensor("w2", [128, KF * D], dt_io, kind="ExternalInput")
    b1r = nc.dram_tensor("b1r", [128, KF], f32, kind="ExternalInput")
    yo = nc.dram_tensor("yo", [128, KD * C], dt_io, kind="ExternalOutput")

    silu = mybir.ActivationFunctionType.Silu

    with tile.TileContext(nc) as tc:
        with (
            tc.tile_pool(name="xp", bufs=1) as xp,
            tc.tile_pool(name="w1p", bufs=4) as w1p,
            tc.tile_pool(name="w2p", bufs=4) as w2p,
            tc.tile_pool(name="hp", bufs=4) as hp,
            tc.tile_pool(name="bp", bufs=1) as bp,
            tc.tile_pool(name="yp", bufs=1) as yp,
            tc.tile_pool(name="sp", bufs=1) as sp,
            tc.tile_pool(name="ps_h", bufs=2, space="PSUM") as ps_h,
            tc.tile_pool(name="ps_y", bufs=1, space="PSUM") as ps_y,
        ):
            # --- PE warm-up: dummy matmuls on memset scratch keep the HAM
            # clock-gate busy during the DMA fill so real matmuls start warm.
            if NDUM > 0:
                wsc = sp.tile([128, 128], dt_io, tag="wsc", name="wsc")
                xsc = sp.tile([128, C], dt_io, tag="xsc", name="xsc")
                nc.vector.memset(wsc[:], 0.0)
                nc.vector.memset(xsc[:], 0.0)
                pdum = ps_h.tile([128, C], f32, tag="hps", name="pdum")
                for _ in range(NDUM):
                    nc.tensor.matmul(
                        pdum[:], wsc[:], xsc[:], start=True, stop=True
                    )

            # --- x first on BOTH rings so it lands before weights saturate
            # HBM (the first matmul needs all of x; weights stream for 30us).
            xw = xp.tile([128, KD * C], dt_io, tag="xw", name="xw")
            nc.sync.dma_start(out=xw[:, :DH], in_=xa[:])
            nc.scalar.dma_start(out=xw[:, DH:], in_=xb[:])
            b1t = bp.tile([128, KF], f32, tag="b1", name="b1t")
            nc.scalar.dma_start(out=b1t[:], in_=b1r[:])

            # --- weight streams (big DMAs; pool bufs pace the queues)
            w1ts: list = []
            f0 = 0
            for g, gw in enumerate(W1G):
                t = w1p.tile(
                    [128, KD * gw * 128],
                    dt_io,
                    tag="w1g",
                    name=f"w1g{g}",
                    padded_shape=[128, KD * max(W1G) * 128],
                )
                off = KD * 128 * f0
                nc.sync.dma_start(out=t[:], in_=w1[:, off : off + KD * gw * 128])
                w1ts.append((f0, gw, t))
                f0 += gw
            w2ts: list = []
            f0 = 0
            for g, gw in enumerate(W2G):
                t = w2p.tile(
                    [128, gw * D],
                    dt_io,
                    tag="w2g",
                    name=f"w2g{g}",
                    padded_shape=[128, max(W2G) * D],
                )
                nc.scalar.dma_start(out=t[:], in_=w2[:, f0 * D : (f0 + gw) * D])
                w2ts.append((f0, t))
                f0 += gw

            w2_of_f = {}
            f0 = 0
            for g, gw in enumerate(W2G):
                for j in range(gw):
                    w2_of_f[f0 + j] = g
                f0 += gw

            py = [
                ps_y.tile([128, C], f32, tag=f"y{dd}", name=f"py{dd}")
                for dd in range(KD)
            ]

            def emit_mm2(f, ht):
                # yT += W2[f-chunk, :]^T @ hT[f-chunk]
                gf0, t = w2ts[w2_of_f[f]]
                r = f - gf0
                for dd in range(KD):
                    nc.tensor.matmul(
                        py[dd][:],
                        t[:, r * D + dd * 128 : r * D + (dd + 1) * 128],
                        ht[:],
                        start=(f == 0),
                        stop=(f == KF - 1),
                    )

            pend = None  # (f, ht) whose mm2 is deferred one chunk
            for gf0, gw, w1t in w1ts:
                for j in range(gw):
                    f = gf0 + j
                    # hT[f-chunk] = silu(sum_d W1[d, f-chunk]^T @ xT[d] + b1)
                    ph = ps_h.tile([128, C], f32, tag="hps", name="ph")
                    for d in range(KD):
                        nc.tensor.matmul(
                            ph[:],
                            w1t[:, (d * gw + j) * 128 : (d * gw + j + 1) * 128],
                            xw[:, d * C : (d + 1) * C],
                            start=(d == 0),
                            stop=(d == KD - 1),
                        )
                    ht = hp.tile([128, C], dt_io, tag="ht", name="ht")
                    nc.scalar.activation(ht[:], ph[:], silu, bias=b1t[:, f : f + 1])
                    # mm2 for the PREVIOUS chunk: its silu ran while this
                    # chunk's mm1 was on the PE, so the PE never waits on ACT
                    if pend is not None:
                        emit_mm2(*pend)
                    pend = (f, ht)
            if pend is not None:
                emit_mm2(*pend)

            # tail: evacuate PSUM on DVE only (an ACT copy would force a
            # second 1.3us ACT-table load onto the scalar queue), casting to
            # dt_io; stream out in one DMA on the (long idle) sync ring
            yt = yp.tile([128, KD * C], dt_io, tag="yt", name="yt")
            for dd in range(KD):
                nc.vector.tensor_copy(yt[:, dd * C : (dd + 1) * C], py[dd][:])
            nc.sync.dma_start(out=yo[:], in_=yt[:])

    nc.compile()
    return nc


def _get_bass(C: int, mode: str, D: int, F: int):
    key = (C, mode, D, F, tuple(W1G), tuple(W2G), NDUM, RAW)
    if key not in _CACHE:
        build = _build_bass_raw if RAW else _build_bass
        _CACHE[key] = build(C, mode, D, F)
    return _CACHE[key]


def _gate_host(x: np.ndarray, Wg: np.ndarray):
    """Top-1 gating in float64: returns (expert_idx [T], gate [T] f32)."""
    logits = x.astype(np.float64) @ Wg.astype(np.float64)
    m = logits.max(-1, keepdims=True)
    p = np.exp(logits - m)
    p /= p.sum(-1, keepdims=True)
    return p.argmax(-1), p.max(-1).astype(np.float32)


def _kernel_numpy(x, Wg, W1, b1, W2, b2):
    """Reference-equivalent fallback (host only)."""
    idx, gate = _gate_host(x, Wg)
    out = np.zeros_like(x)
    for e in range(W1.shape[0]):
        ids = np.nonzero(idx == e)[0]
        if ids.size == 0:
            continue
        h = x[ids] @ W1[e] + b1[e]
        h = h * (1.0 / (1.0 + np.exp(-h)))
        out[ids] = gate[ids, None] * (h @ W2[e] + b2[e])
    return out


def kernel(hidden_states, Wg, W1, b1, W2, b2):
    hidden_states = np.asarray(hidden_states)
    Wg = np.asarray(Wg, dtype=np.float32)
    W1 = np.asarray(W1, dtype=np.float32)
    b1 = np.asarray(b1, dtype=np.float32)
    W2 = np.asarray(W2, dtype=np.float32)
    b2 = np.asarray(b2, dtype=np.float32)

    orig_shape = hidden_states.shape
    D = orig_shape[-1]
    x = np.ascontiguousarray(hidden_states, dtype=np.float32).reshape(-1, D)
    E, _, F = W1.shape
    KD, KF = D // 128, F // 128

    if (
        E != N_CORES
        or D % 128 != 0
        or F % 128 != 0
        or sum(W1G) != KF
        or sum(W2G) != KF
        or KD % 2 != 0
    ):
        return _kernel_numpy(x, Wg, W1, b1, W2, b2).reshape(orig_shape)

    idx, gate = _gate_host(x, Wg)
    order = np.argsort(idx, kind="stable")
    counts = np.bincount(idx, minlength=E)
    starts = np.concatenate([[0], np.cumsum(counts)])

    # Capacity: common padded token count per core (one PSUM-bank slab).
    C = max(256, _roundup(int(counts.max()), 16))
    if C > 512:
        return _kernel_numpy(x, Wg, W1, b1, W2, b2).reshape(orig_shape)

    mode = MODE
    np_io = np.float32
    if mode == "bf16":
        import ml_dtypes

        np_io = ml_dtypes.bfloat16

    nc = _get_bass(C, mode, D, F)

    DH = (KD // 2) * C
    in_maps = []
    for e in range(E):
        ids = order[starts[e] : starts[e + 1]]
        xe = np.zeros((C, D), dtype=np.float32)
        xe[: ids.size] = x[ids]
        # pack per-core images (see _build_bass docstring)
        xr = (
            xe.reshape(C, KD, 128).transpose(2, 1, 0).reshape(128, KD * C)
        )  # [p, d*C+c]
        w1e = W1[e].reshape(KD, 128, KF, 128)
        parts = []
        f0 = 0
        for gw in W1G:
            blk = w1e[:, :, f0 : f0 + gw]  # [KD, 128, gw, 128]
            parts.append(blk.transpose(1, 0, 2, 3).reshape(128, KD * gw * 128))
            f0 += gw
        w1r = np.concatenate(parts, axis=1)  # [128, KD*F]
        w2r = W2[e].reshape(KF, 128, D).transpose(1, 0, 2).reshape(128, KF * D)
        xr = np.ascontiguousarray(xr).astype(np_io, copy=False)
        in_maps.append(
            {
                "xa": np.ascontiguousarray(xr[:, :DH]),
                "xb": np.ascontiguousarray(xr[:, DH:]),
                "w1": np.ascontiguousarray(w1r).astype(np_io, copy=False),
                "w2": np.ascontiguousarray(w2r).astype(np_io, copy=False),
                "b1r": np.ascontiguousarray(b1[e].reshape(KF, 128).T),
            }
        )

    res = run_bass_kernel_spmd(nc, in_maps, list(range(N_CORES)))

    out = np.zeros_like(x)
    for e in range(E):
        ids = order[starts[e] : starts[e + 1]]
        if ids.size == 0:
            continue
        yr = np.asarray(res.results[e]["yo"]).astype(np.float32)  # [128, KD*C]
        y = yr.reshape(128, KD, C).transpose(2, 1, 0).reshape(C, D)[: ids.size]
        out[ids] = gate[ids, None] * (y + b2[e])
    return out.reshape(orig_shape)


# revision 15
# speedup vs baseline: 1.1764x; 1.1764x over previous
"""MoE top-1 feed-forward (DeepSpeed-style) on 8 Trainium2 NeuronCores.

Strategy (expert parallelism, per the sharding hint):
  - Host computes the (tiny) gate: logits = x @ Wg, softmax, top-1 expert id
    and gate prob per token (float64 for a faithful argmax).
  - Tokens are dispatched to the core owning their expert (core e holds
    W1[e]/b1[e]/W2[e]/b2[e]); each core's token batch is padded to a common
    capacity C so all 8 cores run one SPMD program.
  - Each core runs the dense FFN for its tokens in bf16 (tolerance is 2e-2;
    bf16 keeps rel-err ~3e-3 while halving HBM traffic and enabling the
    PE's fast-weight-load path, which fp32 weights cannot use):
        hT = silu(W1^T @ xT + b1);  yT = W2^T @ hT
    with tokens along the free (moving) dimension so no device transposes.
  - Schedule: x is split across BOTH HWDGE rings as the first two DMAs so
    the PE's critical-path input lands before the weight stream saturates
    HBM; a short burst of dummy matmuls on memset scratch warms the PE HAM
    clock-gate during the DMA fill so real matmuls run at 2.4 GHz from the
    start; W1/W2 stream in a few large group DMAs (small leading groups so
    compute starts early); y is evacuated as bf16 and stored in one DMA.
  - Host combines: out[token] = gate * (y + b2[expert]).
"""

import os
import sys

import numpy as np

try:
    import concourse.mybir as mybir  # noqa: F401
except ModuleNotFoundError:  # fallback if the site hooks aren't installed
    sys.path.insert(0, "/opt/trn_rl_repo")

import concourse.mybir as mybir
import concourse.tile as tile
from concourse import bacc
from concourse.bass_utils import run_bass_kernel_spmd

N_CORES = 8

# Compute dtype for the matmuls ("bf16" | "f32r" | "f32").
MODE = os.environ.get("BASS_MOE_MODE", "bf16")

# F-chunk widths per W1 / W2 group DMA (small leads -> compute starts early).
W1G = [int(s) for s in os.environ.get("BASS_MOE_W1G", "1,2,3,6,6,6").split(",")]
W2G = [int(s) for s in os.environ.get("BASS_MOE_W2G", "2,4,6,6,6").split(",")]
NDUM = int(os.environ.get("BASS_MOE_NDUM", "12"))  # PE warm-up matmuls

RAW = os.environ.get("BASS_MOE_RAW", "0") == "1"

_CACHE: dict = {}


def _roundup(a: int, m: int) -> int:
    return -(-a // m) * m


def _build_bass_raw(C: int, mode: str, D: int, F: int):
    """Raw bacc build (no TileContext): hand-rolled semaphores.

    Tile allocates ~254 semaphores and its exit drain serially resets every
    one (~8us of measured kernel time).  This version uses ~18 semaphores
    with explicit waits, full-resident weight buffers (no slot recycling,
    so no WAR waits on weight tiles), and a ~1us tail.

    Semaphore-safety note: a semaphore shared by several HWDGE DMAs may only
    be waited at its FINAL value — each DMA's 16 per-SDMA-engine increments
    interleave across transfers, so partial thresholds can be reached before
    any single transfer is complete.  Hence one semaphore per weight-group
    DMA, each waited at exactly 16.
    """
    import contextlib

    f32 = mybir.dt.float32
    if mode == "bf16":
        dt_io = mybir.dt.bfloat16
    elif mode == "f32r":
        dt_io = mybir.dt.float32r
    else:
        dt_io = f32

    KD, KF = D // 128, F // 128
    assert sum(W1G) == KF and sum(W2G) == KF and C <= 512
    DH = (KD // 2) * C

    nc = bacc.Bacc(None, target_bir_lowering=False, debug=False)
    xa = nc.dram_tensor("xa", [128, DH], dt_io, kind="ExternalInput")
    xb = nc.dram_tensor("xb", [128, KD * C - DH], dt_io, kind="ExternalInput")
    w1 = nc.dram_tensor("w1", [128, KD * F], dt_io, kind="ExternalInput")
    w2 = nc.dram_tensor("w2", [128, KF * D], dt_io, kind="ExternalInput")
    b1r = nc.dram_tensor("b1r", [128, KF], f32, kind="ExternalInput")
    yo = nc.dram_tensor("yo", [128, KD * C], dt_io, kind="ExternalOutput")

    silu = mybir.ActivationFunctionType.Silu
    es = contextlib.ExitStack()
    nc._moe_ctx = es  # keep allocations alive for the life of the program

    sem = lambda name: es.enter_context(nc.semaphore(name))
    s_x = sem("s_x")
    s_b1 = sem("s_b1")
    s_w1 = [sem(f"s_w1_{g}") for g in range(len(W1G))]
    s_w2 = [sem(f"s_w2_{g}") for g in range(len(W2G))]
    s_dum = sem("s_dum")
    s_ph = sem("s_ph")
    s_act = sem("s_act")
    s_mm2 = sem("s_mm2")
    s_fin = sem("s_fin")
    s_dve = sem("s_dve")
    s_yo = sem("s_yo")
    all_sems = [s_x, s_b1, *s_w1, *s_w2, s_dum, s_ph, s_act, s_mm2, s_fin, s_dve, s_yo]

    sb = lambda name, shape, dt: es.enter_context(nc.sbuf_tensor(name, shape, dt))
    ps = lambda name, shape: es.enter_context(nc.psum_tensor(name, shape, f32))
    xw = sb("xw", [128, KD * C], dt_io)
    w1b = sb("w1b", [128, KD * F], dt_io)
    w2b = sb("w2b", [128, KF * D], dt_io)
    b1t = sb("b1t", [128, KF], f32)
    htb = sb("htb", [128, 4 * C], dt_io)  # 4-slot ring of hT chunks
    yt = sb("yt", [128, KD * C], dt_io)
    wsc = sb("wsc", [128, 128], dt_io)
    xsc = sb("xsc", [128, C], dt_io)
    ph = [ps("ph0", [128, C]), ps("ph1", [128, C])]
    py = [ps(f"py{dd}", [128, C]) for dd in range(KD)]

    ht = lambda f: htb[:, (f % 4) * C : (f % 4 + 1) * C]

    # --- DVE: scratch memsets (unblock PE warm-up), later PSUM evacuation
    nc.vector.memset(wsc[:], 0.0).then_inc(s_dum, 1)
    nc.vector.memset(xsc[:], 0.0).then_inc(s_dum, 1)

    w2g_off = [0]
    for gw in W2G:
        w2g_off.append(w2g_off[-1] + gw)

    def w2_load(g):
        f0, gw = w2g_off[g], W2G[g]
        nc.scalar.dma_start(
            out=w2b[:, f0 * D : (f0 + gw) * D], in_=w2[:, f0 * D : (f0 + gw) * D]
        ).then_inc(s_w2[g], 16)

    # --- scalar ring: xb, b1, then only the first two W2 groups — later
    # groups are issued BETWEEN activations (below) so the ACT queue is
    # never blocked behind a long run of DMA issues before the first silu.
    nc.scalar.dma_start(out=xw[:, DH:], in_=xb[:]).then_inc(s_x, 16)
    nc.scalar.dma_start(out=b1t[:], in_=b1r[:]).then_inc(s_b1, 16)
    w2_load(0)
    w2_load(1)

    # --- sync ring: x half first, then the W1 stream
    nc.sync.dma_start(out=xw[:, :DH], in_=xa[:]).then_inc(s_x, 16)
    f0 = 0
    for g, gw in enumerate(W1G):
        off = KD * 128 * f0
        sz = KD * gw * 128
        nc.sync.dma_start(
            out=w1b[:, off : off + sz], in_=w1[:, off : off + sz]
        ).then_inc(s_w1[g], 16)
        f0 += gw

    w1_of_f, w2_of_f = {}, {}
    w1_f0, w2_f0 = {}, {}
    f0 = 0
    for g, gw in enumerate(W1G):
        for j in range(gw):
            w1_of_f[f0 + j] = g
            w1_f0[f0 + j] = f0
        f0 += gw
    f0 = 0
    for g, gw in enumerate(W2G):
        for j in range(gw):
            w2_of_f[f0 + j] = g
        f0 += gw

    # --- PE program
    if NDUM > 0:
        nc.tensor.wait_ge(s_dum, 2)
        for _ in range(NDUM):
            nc.tensor.matmul(ph[0][:], wsc[:], xsc[:], start=True, stop=True)

    def emit_mm2(f):
        g = w2_of_f[f]
        nc.tensor.wait_ge(s_act, f + 1)  # ht(f) written
        nc.tensor.wait_ge(s_w2[g], 16)
        for dd in range(KD):
            mm = nc.tensor.matmul(
                py[dd][:],
                w2b[:, f * D + dd * 128 : f * D + (dd + 1) * 128],
                ht(f),
                start=(f == 0),
                stop=(f == KF - 1),
            )
            if f == KF - 1:
                mm.then_inc(s_fin, 1)
            elif dd == KD - 1:
                mm.then_inc(s_mm2, 1)

    seen_w1 = set()
    for f in range(KF):
        g = w1_of_f[f]
        if g not in seen_w1:
            seen_w1.add(g)
            nc.tensor.wait_ge(s_w1[g], 16)
        if f == 0:
            nc.tensor.wait_ge(s_x, 32)
        if f >= 2:
            nc.tensor.wait_ge(s_act, f - 1)  # ph slot (f-2) consumed
        gw = W1G[g]
        j = f - w1_f0[f]
        off = KD * 128 * w1_f0[f]
        for d in range(KD):
            mm = nc.tensor.matmul(
                ph[f % 2][:],
                w1b[:, off + (d * gw + j) * 128 : off + (d * gw + j + 1) * 128],
                xw[:, d * C : (d + 1) * C],
                start=(d == 0),
                stop=(d == KD - 1),
            )
            if d == KD - 1:
                mm.then_inc(s_ph, 1)
        if f >= 1:
            emit_mm2(f - 1)
    emit_mm2(KF - 1)

    # --- ACT program (after the scalar-ring DMA issues above); remaining
    # W2 group issues are slotted into the silu stream's slack
    for f in range(KF):
        nc.scalar.wait_ge(s_ph, f + 1)
        if f == 0:
            nc.scalar.wait_ge(s_b1, 16)
        if f >= 4:
            nc.scalar.wait_ge(s_mm2, f - 3)  # ht slot (f-4) consumed by mm2
        nc.scalar.activation(
            ht(f), ph[f % 2][:], silu, bias=b1t[:, f : f + 1]
        ).then_inc(s_act, 1)
        g = 2 + (f - 1) // 2  # f==1 -> g2, f==3 -> g3, f==5 -> g4
        if f in (1, 3, 5) and g < len(W2G):
            w2_load(g)

    # --- DVE evacuation (casts f32 PSUM -> dt_io); each copy chases the
    # matching final-chunk matmul so the first half of y can ship early
    for dd in range(KD):
        nc.vector.wait_ge(s_fin, dd + 1)
        nc.vector.tensor_copy(yt[:, dd * C : (dd + 1) * C], py[dd][:]).then_inc(
            s_dve, 1
        )

    # --- y out split across both rings: first half overlaps the last
    # matmuls/casts, second half follows immediately
    half = KD // 2
    nc.sync.wait_ge(s_dve, half)
    nc.sync.dma_start(out=yo[:, : half * C], in_=yt[:, : half * C]).then_inc(s_yo, 16)
    nc.scalar.wait_ge(s_dve, KD)
    nc.scalar.dma_start(out=yo[:, half * C :], in_=yt[:, half * C :]).then_inc(
        s_yo, 16
    )
    nc.sync.wait_ge(s_yo, 32)

    # --- exit: barrier, then reset our semaphores so the NEFF can re-run
    nc.all_engine_barrier()
    nums = sorted(s.num for s in all_sems)
    lo = 0
    while lo < len(nums):
        hi = lo
        while hi + 1 < len(nums) and nums[hi + 1] == nums[hi] + 1:
            hi += 1
        r = range(nums[lo], nums[hi] + 1)
        nc.gpsimd.dma_reset(r)
        nc.gpsimd.sem_clear(r)
        lo = hi + 1
    nc.all_engine_barrier()

    nc.compile()
    return nc


def _build_bass(C: int, mode: str, D: int, F: int):
    """Build + compile the per-core Bass program for capacity C (<= 512)."""
    f32 = mybir.dt.float32
    if mode == "bf16":
        dt_io = mybir.dt.bfloat16
    elif mode == "f32r":
        dt_io = mybir.dt.float32r
    else:
        dt_io = f32
    esz = 2 if mode == "bf16" else 4

    KD, KF = D // 128, F // 128
    assert sum(W1G) == KF and sum(W2G) == KF
    assert C <= 512
    DH = (KD // 2) * C  # x half-image width (d-chunks 0..KD/2-1)

    nc = bacc.Bacc(None, target_bir_lowering=False, debug=False)
    # Host-packed images (see kernel() for packing). Two HWDGE rings: sync
    # carries xa + W1 + y-out, scalar carries xb + b1 + W2.
    #   xa/xb [128, DH]     x^T halves: [p, d*C+c] = x[c, d*128+p]
    #   w1    [128, KD*F]   flat group images; group g at column offset
    #                       KD*128*sum(W1G[:g]), block (d, j) at (d*gw+j)*128
    #   w2    [128, KF*D]   flat image: [p, f*D+j] = W2[f*128+p, j]
    #   b1r   [128, KF]     b1[f*128+p] at [p, f]
    #   yo    [128, KD*C]   output (dt_io): [p, d*C+c] = y[c, d*128+p]
    xa = nc.dram_tensor("xa", [128, DH], dt_io, kind="ExternalInput")
    xb = nc.dram_tensor("xb", [128, KD * C - DH], dt_io, kind="ExternalInput")
    w1 = nc.dram_tensor("w1", [128, KD * F], dt_io, kind="ExternalInput")
    w2 = nc.dram_tensor("w2", [128, KF * D], dt_io, kind="ExternalInput")
    b1r = nc.dram_tensor("b1r", [128, KF], f32, kind="ExternalInput")
    yo = nc.dram_tensor("yo", [128, KD * C], dt_io, kind="ExternalOutput")

    silu = mybir.ActivationFunctionType.Silu

    with tile.TileContext(nc) as tc:
        with (
            tc.tile_pool(name="xp", bufs=1) as xp,
            tc.tile_pool(name="w1p", bufs=4) as w1p,
            tc.tile_pool(name="w2p", bufs=4) as w2p,
            tc.tile_pool(name="hp", bufs=4) as hp,
            tc.tile_pool(name="bp", bufs=1) as bp,
            tc.tile_pool(name="yp", bufs=1) as yp,
            tc.tile_pool(name="sp", bufs=1) as sp,
            tc.tile_pool(name="ps_h", bufs=2, space="PSUM") as ps_h,
            tc.tile_pool(name="ps_y", bufs=1, space="PSUM") as ps_y,
        ):
            # --- PE warm-up: dummy matmuls on memset scratch keep the HAM
            # clock-gate busy during the DMA fill so real matmuls start warm.
            if NDUM > 0:
                wsc = sp.tile([128, 128], dt_io, tag="wsc", name="wsc")
                xsc = sp.tile([128, C], dt_io, tag="xsc", name="xsc")
                nc.vector.memset(wsc[:], 0.0)
                nc.vector.memset(xsc[:], 0.0)
                pdum = ps_h.tile([128, C], f32, tag="hps", name="pdum")
                for _ in range(NDUM):
                    nc.tensor.matmul(
                        pdum[:], wsc[:], xsc[:], start=True, stop=True
                    )

            # --- x first on BOTH rings so it lands before weights saturate
            # HBM (the first matmul needs all of x; weights stream for 30us).
            xw = xp.tile([128, KD * C], dt_io, tag="xw", name="xw")
            nc.sync.dma_start(out=xw[:, :DH], in_=xa[:])
            nc.scalar.dma_start(out=xw[:, DH:], in_=xb[:])
            b1t = bp.tile([128, KF], f32, tag="b1", name="b1t")
            nc.scalar.dma_start(out=b1t[:], in_=b1r[:])

            # --- weight streams (big DMAs; pool bufs pace the queues)
            w1ts: list = []
            f0 = 0
            for g, gw in enumerate(W1G):
                t = w1p.tile(
                    [128, KD * gw * 128],
                    dt_io,
                    tag="w1g",
                    name=f"w1g{g}",
                    padded_shape=[128, KD * max(W1G) * 128],
                )
                off = KD * 128 * f0
                nc.sync.dma_start(out=t[:], in_=w1[:, off : off + KD * gw * 128])
                w1ts.append((f0, gw, t))
                f0 += gw
            w2ts: list = []
            f0 = 0
            for g, gw in enumerate(W2G):
                t = w2p.tile(
                    [128, gw * D],
                    dt_io,
                    tag="w2g",
                    name=f"w2g{g}",
                    padded_shape=[128, max(W2G) * D],
                )
                nc.scalar.dma_start(out=t[:], in_=w2[:, f0 * D : (f0 + gw) * D])
                w2ts.append((f0, t))
                f0 += gw

            w2_of_f = {}
            f0 = 0
            for g, gw in enumerate(W2G):
                for j in range(gw):
                    w2_of_f[f0 + j] = g
                f0 += gw

            py = [
                ps_y.tile([128, C], f32, tag=f"y{dd}", name=f"py{dd}")
                for dd in range(KD)
            ]

            def emit_mm2(f, ht):
                # yT += W2[f-chunk, :]^T @ hT[f-chunk]
                gf0, t = w2ts[w2_of_f[f]]
                r = f - gf0
                for dd in range(KD):
                    nc.tensor.matmul(
                        py[dd][:],
                        t[:, r * D + dd * 128 : r * D + (dd + 1) * 128],
                        ht[:],
                        start=(f == 0),
                        stop=(f == KF - 1),
                    )

            pend = None  # (f, ht) whose mm2 is deferred one chunk
            for gf0, gw, w1t in w1ts:
                for j in range(gw):
                    f = gf0 + j
                    # hT[f-chunk] = silu(sum_d W1[d, f-chunk]^T @ xT[d] + b1)
                    ph = ps_h.tile([128, C], f32, tag="hps", name="ph")
                    for d in range(KD):
                        nc.tensor.matmul(
                            ph[:],
                            w1t[:, (d * gw + j) * 128 : (d * gw + j + 1) * 128],
                            xw[:, d * C : (d + 1) * C],
                            start=(d == 0),
                            stop=(d == KD - 1),
                        )
                    ht = hp.tile([128, C], dt_io, tag="ht", name="ht")
                    nc.scalar.activation(ht[:], ph[:], silu, bias=b1t[:, f : f + 1])
                    # mm2 for the PREVIOUS chunk: its silu ran while this
                    # chunk's mm1 was on the PE, so the PE never waits on ACT
                    if pend is not None:
                        emit_mm2(*pend)
                    pend = (f, ht)
            if pend is not None:
                emit_mm2(*pend)

            # tail: evacuate PSUM on DVE only (an ACT copy would force a
            # second 1.3us ACT-table load onto the scalar queue), casting to
            # dt_io; stream out in one DMA on the (long idle) sync ring
            yt = yp.tile([128, KD * C], dt_io, tag="yt", name="yt")
            for dd in range(KD):
                nc.vector.tensor_copy(yt[:, dd * C : (dd + 1) * C], py[dd][:])
            nc.sync.dma_start(out=yo[:], in_=yt[:])

    nc.compile()
    return nc


def _get_bass(C: int, mode: str, D: int, F: int):
    key = (C, mode, D, F, tuple(W1G), tuple(W2G), NDUM, RAW)
    if key not in _CACHE:
        build = _build_bass_raw if RAW else _build_bass
        _CACHE[key] = build(C, mode, D, F)
    return _CACHE[key]


def _gate_host(x: np.ndarray, Wg: np.ndarray):
    """Top-1 gating in float64: returns (expert_idx [T], gate [T] f32)."""
    logits = x.astype(np.float64) @ Wg.astype(np.float64)
    m = logits.max(-1, keepdims=True)
    p = np.exp(logits - m)
    p /= p.sum(-1, keepdims=True)
    return p.argmax(-1), p.max(-1).astype(np.float32)


def _kernel_numpy(x, Wg, W1, b1, W2, b2):
    """Reference-equivalent fallback (host only)."""
    idx, gate = _gate_host(x, Wg)
    out = np.zeros_like(x)
    for e in range(W1.shape[0]):
        ids = np.nonzero(idx == e)[0]
        if ids.size == 0:
            continue
        h = x[ids] @ W1[e] + b1[e]
        h = h * (1.0 / (1.0 + np.exp(-h)))
        out[ids] = gate[ids, None] * (h @ W2[e] + b2[e])
    return out


def kernel(hidden_states, Wg, W1, b1, W2, b2):
    hidden_states = np.asarray(hidden_states)
    Wg = np.asarray(Wg, dtype=np.float32)
    W1 = np.asarray(W1, dtype=np.float32)
    b1 = np.asarray(b1, dtype=np.float32)
    W2 = np.asarray(W2, dtype=np.float32)
    b2 = np.asarray(b2, dtype=np.float32)

    orig_shape = hidden_states.shape
    D = orig_shape[-1]
    x = np.ascontiguousarray(hidden_states, dtype=np.float32).reshape(-1, D)
    E, _, F = W1.shape
    KD, KF = D // 128, F // 128

    if (
        E != N_CORES
        or D % 128 != 0
        or F % 128 != 0
        or sum(W1G) != KF
        or sum(W2G) != KF
        or KD % 2 != 0
    ):
        return _kernel_numpy(x, Wg, W1, b1, W2, b2).reshape(orig_shape)

    idx, gate = _gate_host(x, Wg)
    order = np.argsort(idx, kind="stable")
    counts = np.bincount(idx, minlength=E)
    starts = np.concatenate([[0], np.cumsum(counts)])

    # Capacity: common padded token count per core (one PSUM-bank slab).
    C = max(256, _roundup(int(counts.max()), 16))
    if C > 512:
        return _kernel_numpy(x, Wg, W1, b1, W2, b2).reshape(orig_shape)

    mode = MODE
    np_io = np.float32
    if mode == "bf16":
        import ml_dtypes

        np_io = ml_dtypes.bfloat16

    nc = _get_bass(C, mode, D, F)

    DH = (KD // 2) * C
    in_maps = []
    for e in range(E):
        ids = order[starts[e] : starts[e + 1]]
        xe = np.zeros((C, D), dtype=np.float32)
        xe[: ids.size] = x[ids]
        # pack per-core images (see _build_bass docstring)
        xr = (
            xe.reshape(C, KD, 128).transpose(2, 1, 0).reshape(128, KD * C)
        )  # [p, d*C+c]
        w1e = W1[e].reshape(KD, 128, KF, 128)
        parts = []
        f0 = 0
        for gw in W1G:
            blk = w1e[:, :, f0 : f0 + gw]  # [KD, 128, gw, 128]
            parts.append(blk.transpose(1, 0, 2, 3).reshape(128, KD * gw * 128))
            f0 += gw
        w1r = np.concatenate(parts, axis=1)  # [128, KD*F]
        w2r = W2[e].reshape(KF, 128, D).transpose(1, 0, 2).reshape(128, KF * D)
        xr = np.ascontiguousarray(xr).astype(np_io, copy=False)
        in_maps.append(
            {
                "xa": np.ascontiguousarray(xr[:, :DH]),
                "xb": np.ascontiguousarray(xr[:, DH:]),
                "w1": np.ascontiguousarray(w1r).astype(np_io, copy=False),
                "w2": np.ascontiguousarray(w2r).astype(np_io, copy=False),
                "b1r": np.ascontiguousarray(b1[e].reshape(KF, 128).T),
            }
        )

    res = run_bass_kernel_spmd(nc, in_maps, list(range(N_CORES)))

    out = np.zeros_like(x)
    for e in range(E):
        ids = order[starts[e] : starts[e + 1]]
        if ids.size == 0:
            continue
        yr = np.asarray(res.results[e]["yo"]).astype(np.float32)  # [128, KD*C]
        y = yr.reshape(128, KD, C).transpose(2, 1, 0).reshape(C, D)[: ids.size]
        out[ids] = gate[ids, None] * (y + b2[e])
    return out.reshape(orig_shape)
